# Initial kernel scaffold
#
"""Trainium2 Bass kernel for attention with per-head qk-layernorm.

Problem (hardcoded): B=2, N=4096, C=1024, H=16, D=64, f32 I/O.
  qkv = x @ qkv_w.T + qkv_b ; per-head LN(q), LN(k) (eps 1e-5)
  attn = softmax(q*D^-0.5 @ k.T) @ v ; out = attn @ proj_w.T + proj_b
Sharding (8 cores): core c -> batch b=c//4, query rows [1024*(c%4), +1024).
Each core computes q,k,v for its own 1024 rows (all 16 heads), AllGathers
kT/v across its 4-core batch group, runs flash attention for its query rows
over the full 4096-key sequence, and projects. Output needs no collective.

Numerics: matmuls bf16 with f32 PSUM accumulation. Softmax skips
max-subtraction (LN bounds |S|<=8). Denominators come from a ones column
appended to V (row 64 of the PV accumulator).

v3 structure:
 - A1: per row tile, k/v qkv chunks + k-LN + kT/v ship + AllGather
   (software pipelined). QKV bias via ones-row matmul (PE), PSUM->SBUF
   chunk copies on the ACT engine -> DVE only does LN math.
 - A2: q chunks + q-LN + transposes for all tiles.
 - Transposes: when q/k norm is identity (w==1, b==0, the common case)
   use the DMA crossbar transpose (frees PE+DVE); otherwise PE transpose
   + DVE affine. Selected at runtime per input values.
 - C: flash attention, hp outer / m inner. S matmuls alternate PE row
   groups (hh0 at partitions 0:64, hh1 at 64:128) -> row-tiled
   concurrency hides every second S matmul. exp in F=1536 chunks.
   A single PV pipeline runs ACROSS (hp,m) boundaries: o-PSUM is
   evacuated (unnormalized + denominator) right after each (hp,m)'s
   last PV, and the reciprocal+broadcast+multiply normalization runs
   in SBUF off the critical path while the next (hp,m) computes.
 - D: projection (bias added during PSUM evacuation; per-tile out DMA).
"""

import os
import sys

for _p in ("/opt/trn_rl_repo", "/root/.axon_site/_ro/trn_rl_repo"):
    if os.path.isdir(_p) and _p not in sys.path:
        sys.path.insert(0, _p)

import numpy as np
import ml_dtypes

B, N, C = 2, 4096, 1024
H, D = 16, 64
NLOC = N // 4          # query rows per core = 1024
P = 128                # partitions
LN_EPS = 1e-5
SCALE = D ** -0.5
N_CORES = 8
BF16 = ml_dtypes.bfloat16

_COMPILED = {}


def build_graph(no_affine):
    import concourse.bass as bass
    import concourse.mybir as mybir
    import concourse.tile as tile
    from concourse import bacc
    from concourse.masks import make_identity

    fp32 = mybir.dt.float32
    bf16 = mybir.dt.bfloat16
    AF = mybir.ActivationFunctionType
    ALU = mybir.AluOpType
    AX = mybir.AxisListType

    nc = bacc.Bacc(trn_type="TRN2", target_bir_lowering=False, num_devices=N_CORES)

    # ---- I/O -------------------------------------------------------------
    xT = nc.declare_dram_parameter("xT", [C, NLOC], bf16, isOutput=False)
    wqkvT = nc.declare_dram_parameter("wqkvT", [C, 3 * C], bf16, isOutput=False)
    qkvb = nc.declare_dram_parameter("qkvb", [1, 3 * C], fp32, isOutput=False)
    wpT = nc.declare_dram_parameter("wpT", [C, C], bf16, isOutput=False)
    pb = nc.declare_dram_parameter("pb", [1, C], fp32, isOutput=False)
    qn_wb = nc.declare_dram_parameter("qn_wb", [D, 2], fp32, isOutput=False)
    kn_wb = nc.declare_dram_parameter("kn_wb", [D, 2], fp32, isOutput=False)
    out = nc.declare_dram_parameter("out", [NLOC, C], fp32, isOutput=True)

    NT = NLOC // P        # 8 local row tiles
    HP = H // 2           # 8 head pairs
    SL = 2 * (D + 1)      # 130: [vA(64)|1|vB(64)|1] per key tile in va
    rg = [[0, 1, 2, 3], [4, 5, 6, 7]]
    JKV = [2, 3, 4, 5]    # k then v qkv channel chunks
    JQ = [0, 1]

    with tile.TileContext(nc) as tc:
        with (
            tc.tile_pool(name="const", bufs=1) as const,
            tc.tile_pool(name="persist", bufs=1) as persist,
            tc.tile_pool(name="dram", bufs=1, space="DRAM") as dram,
        ):
            ident = const.tile([P, P], bf16, tag="ident", name="ident")
            make_identity(nc, ident)
            ones_row = const.tile([1, P], bf16, tag="ones_row", name="ones_row")
            nc.any.memset(ones_row[:], 1.0)
            eps_t = const.tile([P, 1], fp32, tag="eps_t", name="eps_t")
            nc.any.memset(eps_t[:], LN_EPS)

            qkvb_f = const.tile([1, 3 * C], fp32, tag="qkvb_f", name="qkvb_f")
            nc.sync.dma_start(qkvb_f[:], qkvb[:])
            qkvb_bf = const.tile([1, 3 * C], bf16, tag="qkvb_bf", name="qkvb_bf")
            nc.vector.tensor_copy(qkvb_bf[:], qkvb_f[:])
            pb_f = const.tile([1, C], fp32, tag="pb_f", name="pb_f")
            nc.sync.dma_start(pb_f[:], pb[:])
            pb_bc = const.tile([P, C], fp32, tag="pb_bc", name="pb_bc")
            nc.gpsimd.partition_broadcast(pb_bc[:], pb_f[:], channels=P)
            qnwb2 = const.tile([P, 2], fp32, tag="qnwb2", name="qnwb2")
            nc.sync.dma_start(qnwb2[0:D, :], qn_wb[:])
            nc.sync.dma_start(qnwb2[D:2 * D, :], qn_wb[:])
            knwb2 = const.tile([P, 2], fp32, tag="knwb2", name="knwb2")
            nc.sync.dma_start(knwb2[0:D, :], kn_wb[:])
            nc.sync.dma_start(knwb2[D:2 * D, :], kn_wb[:])

            qT_sb = [persist.tile([P, NLOC], bf16, tag=f"qT{p}", name=f"qT{p}") for p in range(HP)]
            attnT = [persist.tile([P, NLOC], bf16, tag=f"aT{p}", name=f"aT{p}") for p in range(HP)]
            # proj weights preloaded early so phase D never waits on DMA
            wp_sb = [persist.tile([P, C], bf16, tag=f"wp{i}", name=f"wp{i}")
                     for i in range(8)]
            for i in range(8):
                nc.sync.dma_start(wp_sb[i][:], wpT[i * P:(i + 1) * P, :])

            kv_loc = [dram.tile([256, C], bf16, tag=f"kvl{i}", name=f"kvl{i}")
                      for i in range(NT)]
            kv_ful = [dram.tile([1024, C], bf16, tag=f"kvf{i}", name=f"kvf{i}")
                      for i in range(NT)]

            # warmup collective: absorb the mesh-algo init (~25us) before the
            # first real AllGather needs it
            dmy_in = dram.tile([1, 64], bf16, tag="dmy_i", name="dmy_i")
            dmy_out = dram.tile([4, 64], bf16, tag="dmy_o", name="dmy_o")
            nc.sync.dma_start(dmy_in[:], ident[0:1, 0:64])
            nc.gpsimd.collective_compute(
                "AllGather", mybir.AluOpType.bypass, replica_groups=rg,
                ins=[dmy_in[:].opt()], outs=[dmy_out[:].opt()])

            with (
                tc.tile_pool(name="qkv_ps", bufs=4, space="PSUM") as qkv_ps,
                tc.tile_pool(name="tp_ps", bufs=3, space="PSUM") as tp_ps,
                tc.tile_pool(name="ln", bufs=2) as ln_pool,
                tc.tile_pool(name="kv_stage", bufs=2) as kv_stage,
                tc.tile_pool(name="pa_w", bufs=1) as pa_w,
            ):
                xT_sb = [pa_w.tile([P, NLOC], bf16, tag=f"xT{i}", name=f"xT{i}") for i in range(8)]
                for i in range(8):
                    nc.sync.dma_start(xT_sb[i][:], xT[i * P:(i + 1) * P, :])
                wq_sb = [pa_w.tile([P, 3 * C], bf16, tag=f"wq{i}", name=f"wq{i}") for i in range(8)]
                # j-major so the first chunk's weights (all 8 kk slices) land
                # first and the first QKV matmul chain starts ~25us earlier
                for j in JKV + JQ:
                    for i in range(8):
                        nc.sync.dma_start(wq_sb[i][:, j * 512:(j + 1) * 512],
                                          wqkvT[i * P:(i + 1) * P, j * 512:(j + 1) * 512])

                def ln_center(t_f, tn, pfx):
                    """tn = (t_f - mu)/std per head (w/b applied post-transpose
                    in the affine variant; identity otherwise). Centering via
                    stride-0 broadcast APs."""
                    t3 = t_f[:].rearrange("p (h d) -> p h d", d=D)
                    sums = ln_pool.tile([P, H], fp32, tag=f"{pfx}sum", name=f"{pfx}sum")
                    nc.vector.tensor_reduce(sums[:], t3, axis=AX.X, op=ALU.add)
                    sq = ln_pool.tile([P, C], fp32, tag=f"{pfx}sq", name=f"{pfx}sq")
                    nc.scalar.activation(sq[:], t_f[:], AF.Square)
                    ssq = ln_pool.tile([P, H], fp32, tag=f"{pfx}ssq", name=f"{pfx}ssq")
                    nc.vector.tensor_reduce(
                        ssq[:], sq[:].rearrange("p (h d) -> p h d", d=D),
                        axis=AX.X, op=ALU.add)
                    mu = ln_pool.tile([P, H], fp32, tag=f"{pfx}mu", name=f"{pfx}mu")
                    nc.vector.tensor_scalar_mul(mu[:], sums[:], 1.0 / D)
                    mu2 = ln_pool.tile([P, H], fp32, tag=f"{pfx}mu2", name=f"{pfx}mu2")
                    nc.vector.tensor_mul(mu2[:], mu[:], mu[:])
                    var = ln_pool.tile([P, H], fp32, tag=f"{pfx}var", name=f"{pfx}var")
                    nc.vector.scalar_tensor_tensor(
                        var[:], ssq[:], 1.0 / D, mu2[:],
                        op0=ALU.mult, op1=ALU.subtract)
                    sig = ln_pool.tile([P, H], fp32, tag=f"{pfx}sig", name=f"{pfx}sig")
                    nc.scalar.activation(sig[:], var[:], AF.Sqrt, bias=eps_t[:])
                    rstd = ln_pool.tile([P, H], fp32, tag=f"{pfx}rstd", name=f"{pfx}rstd")
                    nc.vector.reciprocal(rstd[:], sig[:])
                    cen = ln_pool.tile([P, C], fp32, tag=f"{pfx}cen", name=f"{pfx}cen")
                    cen3 = cen[:].rearrange("p (h d) -> p h d", d=D)
                    tn3 = tn[:].rearrange("p (h d) -> p h d", d=D)
                    mu3 = mu[:].rearrange("p (h o) -> p h o", o=1)
                    rstd3 = rstd[:].rearrange("p (h o) -> p h o", o=1)
                    t3b, mu3b = bass.broadcast_tensor_aps(t3, mu3)
                    nc.vector.tensor_tensor(cen3, t3b, mu3b, op=ALU.subtract)
                    cen3b, rstd3b = bass.broadcast_tensor_aps(cen3, rstd3)
                    nc.vector.tensor_tensor(tn3, cen3b, rstd3b, op=ALU.mult)

                def transpose_blocks(tn, dest_fn, wb2):
                    """dest[hp] <- tn[:, hp*128:(hp+1)*128]^T for all hp.
                    (PE transpose; the PSUM->SBUF copy carries the LN affine.
                    When the norm is identity the affine is a plain copy --
                    same cost -- so a single code path is used.)"""
                    for hp in range(HP):
                        tp = tp_ps.tile([P, P], bf16, tag="tp", name="tp")
                        nc.tensor.transpose(tp[:], tn[:, hp * P:(hp + 1) * P],
                                            ident[:])
                        if no_affine:
                            nc.vector.tensor_copy(dest_fn(hp), tp[:])
                        else:
                            nc.vector.tensor_scalar(
                                dest_fn(hp), tp[:], wb2[:, 0:1], wb2[:, 1:2],
                                op0=ALU.mult, op1=ALU.add)

                def chunk_mms(i, j, dest, dcol):
                    """one 512-col qkv chunk (x@W + bias) -> dest[:, dcol:+512].
                    Bias rides a 1-row matmul; evacuation on the ACT engine."""
                    ps = qkv_ps.tile([P, 512], fp32, tag="ps", name="ps")
                    nc.tensor.matmul(ps[:], ones_row[:, :P],
                                     qkvb_bf[:, j * 512:(j + 1) * 512],
                                     start=True, stop=False)
                    for kk in range(8):
                        nc.tensor.matmul(
                            ps[:],
                            xT_sb[kk][:, i * P:(i + 1) * P],
                            wq_sb[kk][:, j * 512:(j + 1) * 512],
                            start=False, stop=(kk == 7))
                    nc.scalar.activation(dest[:, dcol:dcol + 512], ps[:], AF.Copy)

                def kv_mms(i):
                    k_f = ln_pool.tile([P, C], fp32, tag="k_f", name="k_f")
                    v_bf = kv_stage.tile([P, C], bf16, tag="v_bf", name="v_bf")
                    for j in JKV:
                        if j < 4:
                            chunk_mms(i, j, k_f, (j - 2) * 512)
                        else:
                            chunk_mms(i, j, v_bf, (j - 4) * 512)
                    nc.sync.dma_start(kv_loc[i][P:2 * P, :], v_bf[:])
                    return k_f

                def finish_k(i, k_f):
                    tkn = kv_stage.tile([P, C], bf16, tag="tkn", name="tkn")
                    ln_center(k_f, tkn, "k")
                    kT_stage = kv_stage.tile([P, C], bf16,
                                             tag="kT_stage", name="kT_stage")
                    transpose_blocks(tkn, lambda hp: kT_stage[:, hp * P:(hp + 1) * P],
                                     knwb2)
                    nc.sync.dma_start(kv_loc[i][0:P, :], kT_stage[:])
                    nc.gpsimd.collective_compute(
                        "AllGather", mybir.AluOpType.bypass,
                        replica_groups=rg,
                        ins=[kv_loc[i][:].opt()],
                        outs=[kv_ful[i][:].opt()])

                def q_tile(i):
                    q_f = ln_pool.tile([P, C], fp32, tag="q_f", name="q_f")
                    for j in JQ:
                        chunk_mms(i, j, q_f, j * 512)
                    tqn = ln_pool.tile([P, C], bf16, tag="tqn", name="tqn")
                    ln_center(q_f, tqn, "q")
                    transpose_blocks(tqn, lambda hp: qT_sb[hp][:, i * P:(i + 1) * P],
                                     qnwb2)

                # ---- A1: k/v for all tiles (software-pipelined LN) ----
                prev = None
                for i in range(NT):
                    k_f = kv_mms(i)
                    if prev is not None:
                        finish_k(i - 1, prev)
                    prev = k_f
                finish_k(NT - 1, prev)
                # ---- A2: q for all tiles ----
                for i in range(NT):
                    q_tile(i)

            # ================= Phase C: flash attention =======================
            GRP = 3
            with (
                tc.tile_pool(name="st_ps", bufs=2, space="PSUM") as st_ps,
                tc.tile_pool(name="o_ps", bufs=1, space="PSUM") as o_ps,
                tc.tile_pool(name="kv_sb", bufs=2) as kv_sb,
                tc.tile_pool(name="p_sb", bufs=5) as p_sb,
                tc.tile_pool(name="nrm", bufs=3) as nrm,
            ):
                units_all = [(t, hh) for t in range(32) for hh in range(2)]
                groups = [units_all[g:g + GRP] for g in range(0, len(units_all), GRP)]
                LEAD = 2
                pending = []  # (issue_pv_fn, finalize_fn | None)

                def pump(limit):
                    while len(pending) > limit:
                        fn, fin = pending.pop(0)
                        fn()
                        if fin is not None:
                            fin()

                for hp in range(HP):
                    kT_i = []
                    va_i = []
                    for i in range(8):
                        kt = kv_sb.tile([P, 4 * P], bf16, tag=f"kT{i}", name=f"kT{i}")
                        nc.sync.dma_start(
                            kt[:].rearrange("p (b n) -> p b n", b=4),
                            kv_ful[i][:, hp * P:(hp + 1) * P].rearrange(
                                "(b q p) c -> p b q c", q=2, p=P)[:, :, 0, :])
                        kT_i.append(kt)
                        va = kv_sb.tile([P, 4 * SL], bf16, tag=f"va{i}", name=f"va{i}")
                        nc.vector.memset(va[:, D::(D + 1)], 1.0)
                        for hh in range(2):
                            nc.sync.dma_start(
                                va[:].rearrange("p (b d) -> p b d", d=SL)[
                                    :, :, hh * (D + 1): hh * (D + 1) + D],
                                kv_ful[i][:, hp * P + hh * D: hp * P + (hh + 1) * D
                                          ].rearrange("(b q p) d -> p b q d",
                                                      q=2, p=P)[:, :, 1, :])
                        va_i.append(va)

                    for m in range(2):
                        o_tiles = [o_ps.tile([D + 1, 512], fp32, tag=f"o{hh}",
                                             name=f"o{hh}")
                                   for hh in range(2)]

                        def make_pv(units, p_t, o_tiles=o_tiles, va_i=va_i):
                            def issue():
                                for j, (t, hh) in enumerate(units):
                                    i, b = t // 4, t % 4
                                    nc.tensor.matmul(
                                        o_tiles[hh][:],
                                        va_i[i][:, b * SL + hh * (D + 1):
                                                b * SL + hh * (D + 1) + D + 1],
                                        p_t[:, j * 512:(j + 1) * 512],
                                        start=(t == 0), stop=(t == 31))
                            return issue

                        def make_fin(hp=hp, m=m, o_tiles=o_tiles):
                            def fin():
                                # evacuate o fast, then normalize in SBUF
                                scr = [nrm.tile([D, 512], bf16, tag=f"sc{hh}",
                                                name=f"sc{hh}") for hh in range(2)]
                                lrow = nrm.tile([1, 2 * 512], fp32, tag="lrow",
                                                name="lrow")
                                for hh in range(2):
                                    nc.vector.tensor_copy(scr[hh][:],
                                                          o_tiles[hh][0:D, :])
                                    nc.vector.tensor_copy(
                                        lrow[:, hh * 512:(hh + 1) * 512],
                                        o_tiles[hh][D:D + 1, :])
                                linv = nrm.tile([1, 2 * 512], fp32, tag="linv",
                                                name="linv")
                                nc.vector.reciprocal(linv[:], lrow[:])
                                for hh in range(2):
                                    bc_sb = nrm.tile([D, 512], fp32, tag=f"bs{hh}",
                                                     name=f"bs{hh}")
                                    nc.gpsimd.partition_broadcast(
                                        bc_sb[:], linv[:, hh * 512:(hh + 1) * 512],
                                        channels=D)
                                    nc.vector.tensor_mul(
                                        attnT[hp][hh * D:(hh + 1) * D,
                                                  m * 512:(m + 1) * 512],
                                        scr[hh][:], bc_sb[:])
                            return fin

                        n_groups = len(groups)
                        for gi, units in enumerate(groups):
                            st = st_ps.tile([P, 512 * GRP], fp32, tag="st", name="st")
                            for j, (t, hh) in enumerate(units):
                                i, b = t // 4, t % 4
                                nc.tensor.matmul(
                                    st[:, j * 512:(j + 1) * 512],
                                    kT_i[i][hh * D:(hh + 1) * D, b * P:(b + 1) * P],
                                    qT_sb[hp][hh * D:(hh + 1) * D,
                                              m * 512:(m + 1) * 512],
                                    start=True, stop=True)
                            p_t = p_sb.tile([P, 512 * GRP], bf16, tag="p", name="p")
                            nw = 512 * len(units)
                            nc.scalar.activation(p_t[:, 0:nw], st[:, 0:nw],
                                                 AF.Exp, scale=SCALE)
                            pending.append(
                                (make_pv(units, p_t),
                                 make_fin() if gi == n_groups - 1 else None))
                            # pump every other group -> PV runs of 6 matmuls,
                            # amortizing the PE array reconfig
                            if gi % 2 == 1 or gi == n_groups - 1:
                                pump(LEAD)
                pump(0)

            # ================= Phase D: output projection =====================
            with (
                tc.tile_pool(name="y_ps", bufs=2, space="PSUM") as y_ps,
                tc.tile_pool(name="y_sb", bufs=2) as y_sb_pool,
            ):
                for i in range(NT):
                    y_sb = y_sb_pool.tile([P, C], fp32, tag="y", name="y")
                    for co in range(2):
                        yp = y_ps.tile([P, 512], fp32, tag="yp", name="yp")
                        for p in range(8):
                            nc.tensor.matmul(
                                yp[:],
                                attnT[p][:, i * P:(i + 1) * P],
                                wp_sb[p][:, co * 512:(co + 1) * 512],
                                start=(p == 0), stop=(p == 7))
                        nc.vector.tensor_tensor(
                            y_sb[:, co * 512:(co + 1) * 512], yp[:],
                            pb_bc[:, co * 512:(co + 1) * 512], op=ALU.add)
                    nc.sync.dma_start(out[i * P:(i + 1) * P, :], y_sb[:])

    nc.finalize()
    return nc


def _prep_in_maps(x, qkv_w, qkv_b, q_norm_w, q_norm_b, k_norm_w, k_norm_b,
                  proj_w, proj_b):
    wqkvT = np.ascontiguousarray(qkv_w.T).astype(BF16)
    wpT = np.ascontiguousarray(proj_w.T).astype(BF16)
    qkvb = qkv_b.reshape(1, 3 * C).astype(np.float32)
    pb = proj_b.reshape(1, C).astype(np.float32)
    qn_wb = np.stack([q_norm_w, q_norm_b], axis=1).astype(np.float32)
    kn_wb = np.stack([k_norm_w, k_norm_b], axis=1).astype(np.float32)
    in_maps = []
    for c in range(N_CORES):
        b, s = c // 4, c % 4
        xt = np.ascontiguousarray(x[b, s * NLOC:(s + 1) * NLOC, :].T).astype(BF16)
        in_maps.append({
            "xT": xt, "wqkvT": wqkvT, "qkvb": qkvb, "wpT": wpT, "pb": pb,
            "qn_wb": qn_wb, "kn_wb": kn_wb,
        })
    return in_maps


def _install_ntff_hook_shim():
    """The agent image's antenv lacks axon_hooks; recreate it so trace=True
    can register the NTFF profile hook that trn_boot would have set."""
    import types
    import antenv

    if "antenv.axon_hooks" in sys.modules:
        return
    mod = types.ModuleType("antenv.axon_hooks")
    state = {"fn": None}
    mod.set_axon_ntff_profile_hook = lambda fn: state.__setitem__("fn", fn)
    mod.get_axon_ntff_profile_hook = lambda: state["fn"]
    sys.modules["antenv.axon_hooks"] = mod
    antenv.axon_hooks = mod
    try:
        from trn_agent_boot.trn_boot import _ntff_profile_via_ctypes
        hook = _ntff_profile_via_ctypes("/opt/axon/libaxon_pjrt.so")
        if hook is not None:
            mod.set_axon_ntff_profile_hook(hook)
    except Exception as e:  # degrade to no tracing
        print(f"ntff hook shim failed: {e}", file=sys.stderr)


def kernel(x, qkv_w, qkv_b, q_norm_w, q_norm_b, k_norm_w, k_norm_b,
           proj_w, proj_b, _trace=False):
    from concourse.bass_utils import run_bass_kernel_spmd

    if _trace:
        _install_ntff_hook_shim()

    no_affine = bool(
        np.allclose(q_norm_w, 1.0) and np.allclose(q_norm_b, 0.0)
        and np.allclose(k_norm_w, 1.0) and np.allclose(k_norm_b, 0.0))
    key = ("nc", no_affine)
    if key not in _COMPILED:
        _COMPILED[key] = build_graph(no_affine)
    nc = _COMPILED[key]

    in_maps = _prep_in_maps(x, qkv_w, qkv_b, q_norm_w, q_norm_b,
                            k_norm_w, k_norm_b, proj_w, proj_b)
    res = run_bass_kernel_spmd(nc, in_maps, core_ids=list(range(N_CORES)),
                               trace=_trace)
    out = np.empty((B, N, C), dtype=np.float32)
    for c in range(N_CORES):
        b, s = c // 4, c % 4
        out[b, s * NLOC:(s + 1) * NLOC, :] = res.results[c]["out"]
    if _trace:
        _COMPILED["last_exec_time_ns"] = res.exec_time_ns
        _COMPILED["last_results"] = res
    return out



# revision 1
# speedup vs baseline: 1.0789x; 1.0789x over previous
"""Trainium2 Bass kernel for attention with per-head qk-layernorm.

Problem (hardcoded): B=2, N=4096, C=1024, H=16, D=64, f32 I/O.
  qkv = x @ qkv_w.T + qkv_b ; per-head LN(q), LN(k) (eps 1e-5)
  attn = softmax(q*D^-0.5 @ k.T) @ v ; out = attn @ proj_w.T + proj_b
Sharding (8 cores): core c -> batch b=c//4, query rows [1024*(c%4), +1024).
Each core computes q,k,v for its own 1024 rows (all 16 heads), AllGathers
kT/v across its 4-core batch group, runs flash attention for its query rows
over the full 4096-key sequence, and projects. Output needs no collective.

Numerics: matmuls bf16 with f32 PSUM accumulation. Softmax skips
max-subtraction (LN bounds |S|<=8). Denominators come from a ones column
appended to V (row 64 of the PV accumulator).

v3 structure:
 - A1: per row tile, k/v qkv chunks + k-LN + kT/v ship + AllGather
   (software pipelined). QKV bias via ones-row matmul (PE), PSUM->SBUF
   chunk copies on the ACT engine -> DVE only does LN math.
 - A2: q chunks + q-LN + transposes for all tiles.
 - Transposes: when q/k norm is identity (w==1, b==0, the common case)
   use the DMA crossbar transpose (frees PE+DVE); otherwise PE transpose
   + DVE affine. Selected at runtime per input values.
 - C: flash attention, hp outer / m inner. S matmuls alternate PE row
   groups (hh0 at partitions 0:64, hh1 at 64:128) -> row-tiled
   concurrency hides every second S matmul. exp in F=1536 chunks.
   A single PV pipeline runs ACROSS (hp,m) boundaries: o-PSUM is
   evacuated (unnormalized + denominator) right after each (hp,m)'s
   last PV, and the reciprocal+broadcast+multiply normalization runs
   in SBUF off the critical path while the next (hp,m) computes.
 - D: projection (bias added during PSUM evacuation; per-tile out DMA).
"""

import os
import sys

for _p in ("/opt/trn_rl_repo", "/root/.axon_site/_ro/trn_rl_repo"):
    if os.path.isdir(_p) and _p not in sys.path:
        sys.path.insert(0, _p)

import numpy as np
import ml_dtypes

B, N, C = 2, 4096, 1024
H, D = 16, 64
NLOC = N // 4          # query rows per core = 1024
P = 128                # partitions
LN_EPS = 1e-5
SCALE = D ** -0.5
N_CORES = 8
BF16 = ml_dtypes.bfloat16

_COMPILED = {}


def build_graph(no_affine):
    import concourse.bass as bass
    import concourse.mybir as mybir
    import concourse.tile as tile
    from concourse import bacc
    from concourse.masks import make_identity

    fp32 = mybir.dt.float32
    bf16 = mybir.dt.bfloat16
    AF = mybir.ActivationFunctionType
    ALU = mybir.AluOpType
    AX = mybir.AxisListType

    nc = bacc.Bacc(trn_type="TRN2", target_bir_lowering=False, num_devices=N_CORES)

    # ---- I/O -------------------------------------------------------------
    xT = nc.declare_dram_parameter("xT", [C, NLOC], bf16, isOutput=False)
    wqkvT = nc.declare_dram_parameter("wqkvT", [C, 3 * C], bf16, isOutput=False)
    qkvb = nc.declare_dram_parameter("qkvb", [1, 3 * C], fp32, isOutput=False)
    wpT = nc.declare_dram_parameter("wpT", [C, C], bf16, isOutput=False)
    pb = nc.declare_dram_parameter("pb", [1, C], fp32, isOutput=False)
    qn_wb = nc.declare_dram_parameter("qn_wb", [D, 2], fp32, isOutput=False)
    kn_wb = nc.declare_dram_parameter("kn_wb", [D, 2], fp32, isOutput=False)
    out = nc.declare_dram_parameter("out", [NLOC, C], fp32, isOutput=True)

    NT = NLOC // P        # 8 local row tiles
    HP = H // 2           # 8 head pairs
    SL = 2 * (D + 1)      # 130: [vA(64)|1|vB(64)|1] per key tile in va
    rg = [[0, 1, 2, 3], [4, 5, 6, 7]]
    JKV = [2, 3, 4, 5]    # k then v qkv channel chunks
    JQ = [0, 1]

    with tile.TileContext(nc) as tc:
        with (
            tc.tile_pool(name="const", bufs=1) as const,
            tc.tile_pool(name="persist", bufs=1) as persist,
            tc.tile_pool(name="dram", bufs=1, space="DRAM") as dram,
        ):
            ident = const.tile([P, P], bf16, tag="ident", name="ident")
            make_identity(nc, ident)
            ones_row = const.tile([1, P], bf16, tag="ones_row", name="ones_row")
            nc.any.memset(ones_row[:], 1.0)
            eps_t = const.tile([P, 1], fp32, tag="eps_t", name="eps_t")
            nc.any.memset(eps_t[:], LN_EPS)

            qkvb_f = const.tile([1, 3 * C], fp32, tag="qkvb_f", name="qkvb_f")
            nc.sync.dma_start(qkvb_f[:], qkvb[:])
            qkvb_bf = const.tile([1, 3 * C], bf16, tag="qkvb_bf", name="qkvb_bf")
            nc.vector.tensor_copy(qkvb_bf[:], qkvb_f[:])
            pb_f = const.tile([1, C], fp32, tag="pb_f", name="pb_f")
            nc.sync.dma_start(pb_f[:], pb[:])
            pb_bc = const.tile([P, C], fp32, tag="pb_bc", name="pb_bc")
            nc.gpsimd.partition_broadcast(pb_bc[:], pb_f[:], channels=P)
            qnwb2 = const.tile([P, 2], fp32, tag="qnwb2", name="qnwb2")
            nc.sync.dma_start(qnwb2[0:D, :], qn_wb[:])
            nc.sync.dma_start(qnwb2[D:2 * D, :], qn_wb[:])
            knwb2 = const.tile([P, 2], fp32, tag="knwb2", name="knwb2")
            nc.sync.dma_start(knwb2[0:D, :], kn_wb[:])
            nc.sync.dma_start(knwb2[D:2 * D, :], kn_wb[:])

            qT_sb = [persist.tile([P, NLOC], bf16, tag=f"qT{p}", name=f"qT{p}") for p in range(HP)]
            attnT = [persist.tile([P, NLOC], bf16, tag=f"aT{p}", name=f"aT{p}") for p in range(HP)]
            # proj weights preloaded early so phase D never waits on DMA
            wp_sb = [persist.tile([P, C], bf16, tag=f"wp{i}", name=f"wp{i}")
                     for i in range(8)]
            for i in range(8):
                nc.sync.dma_start(wp_sb[i][:], wpT[i * P:(i + 1) * P, :])

            kv_loc = [dram.tile([256, C], bf16, tag=f"kvl{i}", name=f"kvl{i}")
                      for i in range(NT)]
            kv_ful = [dram.tile([1024, C], bf16, tag=f"kvf{i}", name=f"kvf{i}")
                      for i in range(NT)]

            # warmup collective: absorb the mesh-algo init (~25us) before the
            # first real AllGather needs it
            dmy_in = dram.tile([1, 64], bf16, tag="dmy_i", name="dmy_i")
            dmy_out = dram.tile([4, 64], bf16, tag="dmy_o", name="dmy_o")
            nc.sync.dma_start(dmy_in[:], ident[0:1, 0:64])
            nc.gpsimd.collective_compute(
                "AllGather", mybir.AluOpType.bypass, replica_groups=rg,
                ins=[dmy_in[:].opt()], outs=[dmy_out[:].opt()])

            with (
                tc.tile_pool(name="qkv_ps", bufs=4, space="PSUM") as qkv_ps,
                tc.tile_pool(name="tp_ps", bufs=3, space="PSUM") as tp_ps,
                tc.tile_pool(name="ln", bufs=2) as ln_pool,
                tc.tile_pool(name="kv_stage", bufs=2) as kv_stage,
                tc.tile_pool(name="pa_w", bufs=1) as pa_w,
            ):
                xT_sb = [pa_w.tile([P, NLOC], bf16, tag=f"xT{i}", name=f"xT{i}") for i in range(8)]
                for i in range(8):
                    nc.sync.dma_start(xT_sb[i][:], xT[i * P:(i + 1) * P, :])
                wq_sb = [pa_w.tile([P, 3 * C], bf16, tag=f"wq{i}", name=f"wq{i}") for i in range(8)]
                # j-major so the first chunk's weights (all 8 kk slices) land
                # first and the first QKV matmul chain starts ~25us earlier
                for j in JKV + JQ:
                    for i in range(8):
                        nc.sync.dma_start(wq_sb[i][:, j * 512:(j + 1) * 512],
                                          wqkvT[i * P:(i + 1) * P, j * 512:(j + 1) * 512])

                def ln_center(t_f, tn, pfx):
                    """tn = (t_f - mu)/std per head (w/b applied post-transpose
                    in the affine variant; identity otherwise). Centering via
                    stride-0 broadcast APs."""
                    t3 = t_f[:].rearrange("p (h d) -> p h d", d=D)
                    sums = ln_pool.tile([P, H], fp32, tag=f"{pfx}sum", name=f"{pfx}sum")
                    nc.vector.tensor_reduce(sums[:], t3, axis=AX.X, op=ALU.add)
                    sq = ln_pool.tile([P, C], fp32, tag=f"{pfx}sq", name=f"{pfx}sq")
                    nc.scalar.activation(sq[:], t_f[:], AF.Square)
                    ssq = ln_pool.tile([P, H], fp32, tag=f"{pfx}ssq", name=f"{pfx}ssq")
                    nc.vector.tensor_reduce(
                        ssq[:], sq[:].rearrange("p (h d) -> p h d", d=D),
                        axis=AX.X, op=ALU.add)
                    mu = ln_pool.tile([P, H], fp32, tag=f"{pfx}mu", name=f"{pfx}mu")
                    nc.vector.tensor_scalar_mul(mu[:], sums[:], 1.0 / D)
                    mu2 = ln_pool.tile([P, H], fp32, tag=f"{pfx}mu2", name=f"{pfx}mu2")
                    nc.vector.tensor_mul(mu2[:], mu[:], mu[:])
                    var = ln_pool.tile([P, H], fp32, tag=f"{pfx}var", name=f"{pfx}var")
                    nc.vector.scalar_tensor_tensor(
                        var[:], ssq[:], 1.0 / D, mu2[:],
                        op0=ALU.mult, op1=ALU.subtract)
                    sig = ln_pool.tile([P, H], fp32, tag=f"{pfx}sig", name=f"{pfx}sig")
                    nc.scalar.activation(sig[:], var[:], AF.Sqrt, bias=eps_t[:])
                    rstd = ln_pool.tile([P, H], fp32, tag=f"{pfx}rstd", name=f"{pfx}rstd")
                    nc.vector.reciprocal(rstd[:], sig[:])
                    cen = ln_pool.tile([P, C], fp32, tag=f"{pfx}cen", name=f"{pfx}cen")
                    cen3 = cen[:].rearrange("p (h d) -> p h d", d=D)
                    tn3 = tn[:].rearrange("p (h d) -> p h d", d=D)
                    mu3 = mu[:].rearrange("p (h o) -> p h o", o=1)
                    rstd3 = rstd[:].rearrange("p (h o) -> p h o", o=1)
                    t3b, mu3b = bass.broadcast_tensor_aps(t3, mu3)
                    nc.vector.tensor_tensor(cen3, t3b, mu3b, op=ALU.subtract)
                    cen3b, rstd3b = bass.broadcast_tensor_aps(cen3, rstd3)
                    nc.vector.tensor_tensor(tn3, cen3b, rstd3b, op=ALU.mult)

                def transpose_blocks(tn, dest_fn, wb2):
                    """dest[hp] <- tn[:, hp*128:(hp+1)*128]^T for all hp.
                    (PE transpose; the PSUM->SBUF copy carries the LN affine.
                    When the norm is identity the affine is a plain copy --
                    same cost -- so a single code path is used.)"""
                    for hp in range(HP):
                        tp = tp_ps.tile([P, P], bf16, tag="tp", name="tp")
                        nc.tensor.transpose(tp[:], tn[:, hp * P:(hp + 1) * P],
                                            ident[:])
                        if no_affine:
                            nc.vector.tensor_copy(dest_fn(hp), tp[:])
                        else:
                            nc.vector.tensor_scalar(
                                dest_fn(hp), tp[:], wb2[:, 0:1], wb2[:, 1:2],
                                op0=ALU.mult, op1=ALU.add)

                def chunk_mms(i, j, dest, dcol):
                    """one 512-col qkv chunk (x@W + bias) -> dest[:, dcol:+512].
                    Bias rides a 1-row matmul; evacuation on the ACT engine."""
                    ps = qkv_ps.tile([P, 512], fp32, tag="ps", name="ps")
                    nc.tensor.matmul(ps[:], ones_row[:, :P],
                                     qkvb_bf[:, j * 512:(j + 1) * 512],
                                     start=True, stop=False)
                    for kk in range(8):
                        nc.tensor.matmul(
                            ps[:],
                            xT_sb[kk][:, i * P:(i + 1) * P],
                            wq_sb[kk][:, j * 512:(j + 1) * 512],
                            start=False, stop=(kk == 7))
                    nc.scalar.activation(dest[:, dcol:dcol + 512], ps[:], AF.Copy)

                def kv_mms(i):
                    k_f = ln_pool.tile([P, C], fp32, tag="k_f", name="k_f")
                    v_bf = kv_stage.tile([P, C], bf16, tag="v_bf", name="v_bf")
                    for j in JKV:
                        if j < 4:
                            chunk_mms(i, j, k_f, (j - 2) * 512)
                        else:
                            chunk_mms(i, j, v_bf, (j - 4) * 512)
                    nc.sync.dma_start(kv_loc[i][P:2 * P, :], v_bf[:])
                    return k_f

                def finish_k(i, k_f):
                    tkn = kv_stage.tile([P, C], bf16, tag="tkn", name="tkn")
                    ln_center(k_f, tkn, "k")
                    kT_stage = kv_stage.tile([P, C], bf16,
                                             tag="kT_stage", name="kT_stage")
                    transpose_blocks(tkn, lambda hp: kT_stage[:, hp * P:(hp + 1) * P],
                                     knwb2)
                    nc.sync.dma_start(kv_loc[i][0:P, :], kT_stage[:])
                    nc.gpsimd.collective_compute(
                        "AllGather", mybir.AluOpType.bypass,
                        replica_groups=rg,
                        ins=[kv_loc[i][:].opt()],
                        outs=[kv_ful[i][:].opt()])

                def q_tile(i):
                    q_f = ln_pool.tile([P, C], fp32, tag="q_f", name="q_f")
                    for j in JQ:
                        chunk_mms(i, j, q_f, j * 512)
                    tqn = ln_pool.tile([P, C], bf16, tag="tqn", name="tqn")
                    ln_center(q_f, tqn, "q")
                    transpose_blocks(tqn, lambda hp: qT_sb[hp][:, i * P:(i + 1) * P],
                                     qnwb2)

                # ---- A1: k/v for all tiles (software-pipelined LN) ----
                prev = None
                for i in range(NT):
                    k_f = kv_mms(i)
                    if prev is not None:
                        finish_k(i - 1, prev)
                    prev = k_f
                finish_k(NT - 1, prev)
                # ---- A2: q for all tiles ----
                for i in range(NT):
                    q_tile(i)

            # ================= Phase C: flash attention =======================
            GRP = 3
            with (
                tc.tile_pool(name="st_ps", bufs=2, space="PSUM") as st_ps,
                tc.tile_pool(name="o_ps", bufs=1, space="PSUM") as o_ps,
                tc.tile_pool(name="kv_sb", bufs=2) as kv_sb,
                tc.tile_pool(name="p_sb", bufs=5) as p_sb,
                tc.tile_pool(name="nrm", bufs=3) as nrm,
            ):
                units_all = [(t, hh) for t in range(32) for hh in range(2)]
                groups = [units_all[g:g + GRP] for g in range(0, len(units_all), GRP)]
                LEAD = 2
                pending = []  # (issue_pv_fn, finalize_fn | None)

                def pump(limit):
                    while len(pending) > limit:
                        fn, fin = pending.pop(0)
                        fn()
                        if fin is not None:
                            fin()

                for hp in range(HP):
                    kT_i = []
                    va_i = []
                    for i in range(8):
                        kt = kv_sb.tile([P, 4 * P], bf16, tag=f"kT{i}", name=f"kT{i}")
                        nc.sync.dma_start(
                            kt[:].rearrange("p (b n) -> p b n", b=4),
                            kv_ful[i][:, hp * P:(hp + 1) * P].rearrange(
                                "(b q p) c -> p b q c", q=2, p=P)[:, :, 0, :])
                        kT_i.append(kt)
                        va = kv_sb.tile([P, 4 * SL], bf16, tag=f"va{i}", name=f"va{i}")
                        nc.vector.memset(va[:, D::(D + 1)], 1.0)
                        for hh in range(2):
                            nc.sync.dma_start(
                                va[:].rearrange("p (b d) -> p b d", d=SL)[
                                    :, :, hh * (D + 1): hh * (D + 1) + D],
                                kv_ful[i][:, hp * P + hh * D: hp * P + (hh + 1) * D
                                          ].rearrange("(b q p) d -> p b q d",
                                                      q=2, p=P)[:, :, 1, :])
                        va_i.append(va)

                    for m in range(2):
                        o_tiles = [o_ps.tile([D + 1, 512], fp32, tag=f"o{hh}",
                                             name=f"o{hh}")
                                   for hh in range(2)]

                        def make_pv(units, p_t, o_tiles=o_tiles, va_i=va_i):
                            def issue():
                                for j, (t, hh) in enumerate(units):
                                    i, b = t // 4, t % 4
                                    nc.tensor.matmul(
                                        o_tiles[hh][:],
                                        va_i[i][:, b * SL + hh * (D + 1):
                                                b * SL + hh * (D + 1) + D + 1],
                                        p_t[:, j * 512:(j + 1) * 512],
                                        start=(t == 0), stop=(t == 31))
                            return issue

                        def make_fin(hp=hp, m=m, o_tiles=o_tiles):
                            def fin():
                                # evacuate o fast, then normalize in SBUF
                                scr = [nrm.tile([D, 512], bf16, tag=f"sc{hh}",
                                                name=f"sc{hh}") for hh in range(2)]
                                lrow = nrm.tile([1, 2 * 512], fp32, tag="lrow",
                                                name="lrow")
                                for hh in range(2):
                                    nc.vector.tensor_copy(scr[hh][:],
                                                          o_tiles[hh][0:D, :])
                                    nc.vector.tensor_copy(
                                        lrow[:, hh * 512:(hh + 1) * 512],
                                        o_tiles[hh][D:D + 1, :])
                                linv = nrm.tile([1, 2 * 512], fp32, tag="linv",
                                                name="linv")
                                nc.vector.reciprocal(linv[:], lrow[:])
                                for hh in range(2):
                                    bc_sb = nrm.tile([D, 512], fp32, tag=f"bs{hh}",
                                                     name=f"bs{hh}")
                                    nc.gpsimd.partition_broadcast(
                                        bc_sb[:], linv[:, hh * 512:(hh + 1) * 512],
                                        channels=D)
                                    nc.vector.tensor_mul(
                                        attnT[hp][hh * D:(hh + 1) * D,
                                                  m * 512:(m + 1) * 512],
                                        scr[hh][:], bc_sb[:])
                            return fin

                        n_groups = len(groups)
                        for gi, units in enumerate(groups):
                            st = st_ps.tile([P, 512 * GRP], fp32, tag="st", name="st")
                            for j, (t, hh) in enumerate(units):
                                i, b = t // 4, t % 4
                                nc.tensor.matmul(
                                    st[:, j * 512:(j + 1) * 512],
                                    kT_i[i][hh * D:(hh + 1) * D, b * P:(b + 1) * P],
                                    qT_sb[hp][hh * D:(hh + 1) * D,
                                              m * 512:(m + 1) * 512],
                                    start=True, stop=True)
                            p_t = p_sb.tile([P, 512 * GRP], bf16, tag="p", name="p")
                            nw = 512 * len(units)
                            nc.scalar.activation(p_t[:, 0:nw], st[:, 0:nw],
                                                 AF.Exp, scale=SCALE)
                            pending.append(
                                (make_pv(units, p_t),
                                 make_fin() if gi == n_groups - 1 else None))
                            # pump every other group -> PV runs of 6 matmuls,
                            # amortizing the PE array reconfig
                            if gi % 2 == 1 or gi == n_groups - 1:
                                pump(LEAD)
                pump(0)

            # ================= Phase D: output projection =====================
            with (
                tc.tile_pool(name="y_ps", bufs=2, space="PSUM") as y_ps,
                tc.tile_pool(name="y_sb", bufs=2) as y_sb_pool,
            ):
                for i in range(NT):
                    y_sb = y_sb_pool.tile([P, C], fp32, tag="y", name="y")
                    for co in range(2):
                        yp = y_ps.tile([P, 512], fp32, tag="yp", name="yp")
                        for p in range(8):
                            nc.tensor.matmul(
                                yp[:],
                                attnT[p][:, i * P:(i + 1) * P],
                                wp_sb[p][:, co * 512:(co + 1) * 512],
                                start=(p == 0), stop=(p == 7))
                        nc.vector.tensor_tensor(
                            y_sb[:, co * 512:(co + 1) * 512], yp[:],
                            pb_bc[:, co * 512:(co + 1) * 512], op=ALU.add)
                    nc.sync.dma_start(out[i * P:(i + 1) * P, :], y_sb[:])

    nc.finalize()
    return nc


def _prep_in_maps(x, qkv_w, qkv_b, q_norm_w, q_norm_b, k_norm_w, k_norm_b,
                  proj_w, proj_b):
    wqkvT = np.ascontiguousarray(qkv_w.T).astype(BF16)
    wpT = np.ascontiguousarray(proj_w.T).astype(BF16)
    qkvb = qkv_b.reshape(1, 3 * C).astype(np.float32)
    pb = proj_b.reshape(1, C).astype(np.float32)
    qn_wb = np.stack([q_norm_w, q_norm_b], axis=1).astype(np.float32)
    kn_wb = np.stack([k_norm_w, k_norm_b], axis=1).astype(np.float32)
    in_maps = []
    for c in range(N_CORES):
        b, s = c // 4, c % 4
        xt = np.ascontiguousarray(x[b, s * NLOC:(s + 1) * NLOC, :].T).astype(BF16)
        in_maps.append({
            "xT": xt, "wqkvT": wqkvT, "qkvb": qkvb, "wpT": wpT, "pb": pb,
            "qn_wb": qn_wb, "kn_wb": kn_wb,
        })
    return in_maps


def _install_ntff_hook_shim():
    """The agent image's antenv lacks axon_hooks; recreate it so trace=True
    can register the NTFF profile hook that trn_boot would have set."""
    import types
    import antenv

    if "antenv.axon_hooks" in sys.modules:
        return
    mod = types.ModuleType("antenv.axon_hooks")
    state = {"fn": None}
    mod.set_axon_ntff_profile_hook = lambda fn: state.__setitem__("fn", fn)
    mod.get_axon_ntff_profile_hook = lambda: state["fn"]
    sys.modules["antenv.axon_hooks"] = mod
    antenv.axon_hooks = mod
    try:
        from trn_agent_boot.trn_boot import _ntff_profile_via_ctypes
        hook = _ntff_profile_via_ctypes("/opt/axon/libaxon_pjrt.so")
        if hook is not None:
            mod.set_axon_ntff_profile_hook(hook)
    except Exception as e:  # degrade to no tracing
        print(f"ntff hook shim failed: {e}", file=sys.stderr)


def kernel(x, qkv_w, qkv_b, q_norm_w, q_norm_b, k_norm_w, k_norm_b,
           proj_w, proj_b, _trace=False):
    from concourse.bass_utils import run_bass_kernel_spmd

    if _trace:
        _install_ntff_hook_shim()

    no_affine = bool(
        np.allclose(q_norm_w, 1.0) and np.allclose(q_norm_b, 0.0)
        and np.allclose(k_norm_w, 1.0) and np.allclose(k_norm_b, 0.0))
    key = ("nc", no_affine)
    if key not in _COMPILED:
        _COMPILED[key] = build_graph(no_affine)
    nc = _COMPILED[key]

    in_maps = _prep_in_maps(x, qkv_w, qkv_b, q_norm_w, q_norm_b,
                            k_norm_w, k_norm_b, proj_w, proj_b)
    res = run_bass_kernel_spmd(nc, in_maps, core_ids=list(range(N_CORES)),
                               trace=_trace)
    out = np.empty((B, N, C), dtype=np.float32)
    for c in range(N_CORES):
        b, s = c // 4, c % 4
        out[b, s * NLOC:(s + 1) * NLOC, :] = res.results[c]["out"]
    if _trace:
        _COMPILED["last_exec_time_ns"] = res.exec_time_ns
        _COMPILED["last_results"] = res
    return out

